# revision 14
# baseline (speedup 1.0000x reference)
"""MultiHeadAttention Trainium2 kernel (8-core SPMD).

Problem: B=2, T=2048, C=1024, H=16 heads, D=64.
  out = softmax((q Wq^T + bq)(k Wk^T + bk)^T / sqrt(D)) (v Wv^T + bv) Wo^T + bo

Sharding: core c -> (batch b = c // 4, head-group g = c % 4).  Each core
computes 4 heads (a 256-wide slice of the projection space) of one batch
element, including its partial contribution to the row-sharded output
projection.  The host sums the 4 partial outputs per batch and adds bo
(bo itself is folded on-device into the g==0 core's partial).

Per-core dataflow (all matmuls in float32r = tf32, fp32 accumulate):
  - PE-transpose q/k/v tiles to get channel-major activations (contraction
    over C needs C on partitions).
  - QT/KT/VT = W_s @ x^T  ([256, T] channel-major), bias folded in.
  - V_nat[k, d] from VT via PE transposes (needed as AV stationary operand).
  - S^T[k, q] = K_h Q_h^T per head (scores transposed -> no P transpose
    before AV); exp via ScalarE with scale=1/8 folded in; no max
    subtraction (|scores/8| ~ 2 for these inputs, exp is safe in fp32).
  - Row sums via ones-matmul (32 replicated rows per head), softmax
    normalization deferred to the [256, T] attention output.
  - partial^T[co, t] = Wo_s^T.T @ O^T accumulated over the 256 head dims.
"""

import numpy as np

B, T, C, H, D = 2, 2048, 1024, 16, 64
NCORES = 8
GROUPS = 4              # head-groups == cores per batch element
HG = H // GROUPS        # heads per core
DS = HG * D             # per-core projection slice width (256)
TCH = 512               # token chunk (psum bank = 512 fp32)
NTCH = T // TCH         # 4
NCC = C // 128          # 8 contraction chunks
NKT = T // 128          # 16 key tiles
SCALE = float(D) ** -0.5

_NC_CACHE = None


def _emit(ctx, tc, io):
    import concourse.bass as bass
    from concourse import mybir

    nc = tc.nc
    f32 = mybir.dt.float32
    f32r = mybir.dt.float32r
    bf16 = mybir.dt.bfloat16
    EXP = mybir.ActivationFunctionType.Exp

    def r(ap):
        return ap.bitcast(f32r)

    persist = ctx.enter_context(tc.tile_pool(name="persist", bufs=1))

    def ptile(tag, shape, dt=f32):
        return persist.tile(shape, dt, tag=tag, name=tag)

    # --- persistent SBUF tensors ---------------------------------------
    ident = ptile("ident", [128, 128])
    nc.sync.dma_start(ident[:], io["ident"][:, :])
    ones = ptile("ones", [128, 32], bf16)
    nc.vector.memset(ones[:], 1.0)
    zeros = ptile("zeros", [128, TCH], bf16)
    nc.vector.memset(zeros[:], 0.0)

    wsb = {}
    for name in ("wqt", "wkt", "wvt"):
        tiles = []
        for c in range(NCC):
            t_ = ptile(f"{name}{c}", [128, DS], f32r)
            nc.sync.dma_start(t_[:], io[name][c * 128:(c + 1) * 128, :])
            tiles.append(t_)
        wsb[name] = tiles
    wot = []
    for dc in range(2):
        t_ = ptile(f"wot{dc}", [128, C], f32r)
        nc.sync.dma_start(t_[:], io["wot"][dc * 128:(dc + 1) * 128, :])
        wot.append(t_)

    bias = {}
    for name, width in (("bqs", 2), ("bks", 2), ("bvs", 2), ("bos", 8)):
        t_ = ptile(name, [128, width])
        nc.sync.dma_start(
            t_[:], io[name].rearrange("(a p) o -> p (a o)", p=128))
        bias[name] = t_

    # channel-major projected activations: 2 tiles of [128, T] each
    QT = [ptile(f"qt{i}", [128, T], f32r) for i in range(2)]
    KT = [ptile(f"kt{i}", [128, T], f32r) for i in range(2)]
    VT = [ptile(f"vt{i}", [128, T], f32r) for i in range(2)]
    VN = [ptile(f"vn{i}", [128, DS], bf16) for i in range(NKT)]   # V natural [k, d]

    # --- stage A: transpose + project q, k, v --------------------------
    with tc.tile_pool(name="nat", bufs=6) as natp, \
         tc.tile_pool(name="xtsb", bufs=10) as xtsbp, \
         tc.tile_pool(name="xtps", bufs=2, space="PSUM") as xtps, \
         tc.tile_pool(name="projps", bufs=2, space="PSUM") as projps:

        def stage_a(xname, wname, bname, XT):
            for tci in range(NTCH):
                nat = []
                for j in range(4):
                    nt = natp.tile([128, C], f32, tag="nat", name="nat")
                    t0 = tci * TCH + j * 128
                    nc.sync.dma_start(nt[:], io[xname][t0:t0 + 128, :])
                    nat.append(nt)
                xts = []
                for c in range(NCC):
                    ps = xtps.tile([128, TCH], f32, tag="xt", name="xtps")
                    for j in range(4):
                        nc.tensor.matmul(
                            ps[:, j * 128:(j + 1) * 128],
                            lhsT=nat[j][:, c * 128:(c + 1) * 128],
                            rhs=ident[:],
                            is_transpose=True,
                            start=(j == 0), stop=(j == 3))
                    xt = xtsbp.tile([128, TCH], f32r, tag="xt", name="xtsb")
                    if c % 2 == 0:
                        nc.scalar.copy(xt[:], ps[:])
                    else:
                        nc.vector.tensor_copy(xt[:], ps[:])
                    xts.append(xt)
                for co in range(2):
                    pj = projps.tile([128, TCH], f32, tag="proj", name="proj")
                    for c in range(NCC):
                        nc.tensor.matmul(
                            pj[:],
                            lhsT=r(wsb[wname][c][:, co * 128:(co + 1) * 128]),
                            rhs=r(xts[c][:]),
                            start=(c == 0), stop=(c == NCC - 1))
                    nc.vector.tensor_scalar_add(
                        XT[co][:, tci * TCH:(tci + 1) * TCH],
                        pj[:], bias[bname][:, co:co + 1])

        stage_a("xk", "wkt", "bks", KT)
        stage_a("xv", "wvt", "bvs", VT)
        stage_a("xq", "wqt", "bqs", QT)

        # V natural [k, d] tiles from VT via PE transposes
        for tp in range(NKT // 2):
            ps = xtps.tile([128, TCH], f32, tag="xt", name="xtps")
            for u in range(2):
                tt = 2 * tp + u
                for dc in range(2):
                    q_ = 2 * u + dc
                    nc.tensor.matmul(
                        ps[:, q_ * 128:(q_ + 1) * 128],
                        lhsT=VT[dc][:, tt * 128:(tt + 1) * 128].bitcast(f32),
                        rhs=ident[:],
                        is_transpose=True,
                        start=(q_ == 0), stop=(q_ == 3))
            nc.scalar.copy(VN[2 * tp][:], ps[:, 0:DS])
            nc.vector.tensor_copy(VN[2 * tp + 1][:], ps[:, DS:2 * DS])

    # --- stage B/C: attention + output projection ----------------------
    with tc.tile_pool(name="sps", bufs=2, space="PSUM") as sps, \
         tc.tile_pool(name="otps", bufs=2, space="PSUM") as otps, \
         tc.tile_pool(name="sumsps", bufs=1, space="PSUM") as sumsps, \
         tc.tile_pool(name="prjps", bufs=1, space="PSUM") as prjps, \
         tc.tile_pool(name="expsb", bufs=4) as expsb, \
         tc.tile_pool(name="otsb", bufs=4) as otsbp, \
         tc.tile_pool(name="recsb", bufs=2) as recp, \
         tc.tile_pool(name="outsb", bufs=3) as outsbp:

        for qc in range(NTCH):
            qcols = slice(qc * TCH, (qc + 1) * TCH)
            sums = sumsps.tile([128, TCH], f32, tag="sums", name="sums")
            otp = [otps.tile([128, TCH], f32, tag="ot", name="ot") for _ in range(2)]
            # open each accumulator bank with one full-partition zeroing
            # matmul so the per-head partial-partition matmuls can all be
            # accumulating (start=False) -- a bank can only host one
            # accumulation group at a time.
            for acc in (otp[0], otp[1], sums):
                nc.tensor.matmul(acc[:, :], lhsT=zeros[:, 0:128],
                                 rhs=zeros[:], start=True, stop=False,
                                 skip_group_check=True)
            for g in range(NKT // 2):
                for h in range(HG):
                    pr, hh = divmod(h, 2)
                    rows = slice(hh * 64, (hh + 1) * 64)
                    S = sps.tile([128, 2 * TCH], f32, tag="s", name="s")
                    for j in range(2):
                        kt = 2 * g + j
                        nc.tensor.matmul(
                            S[:, j * TCH:(j + 1) * TCH],
                            lhsT=r(KT[pr][rows, kt * 128:(kt + 1) * 128]),
                            rhs=r(QT[pr][rows, qcols]),
                            start=True, stop=True)
                    es = expsb.tile([128, 2 * TCH], bf16, tag="es", name="es")
                    nc.scalar.activation(es[:], S[:], EXP, scale=SCALE)
                    last = (g == NKT // 2 - 1)
                    for j in range(2):
                        kt = 2 * g + j
                        ej = es[:, j * TCH:(j + 1) * TCH]
                        # each head's partition slice closes its own group
                        # on its final accumulating matmul
                        nc.tensor.matmul(
                            otp[pr][rows, :],
                            lhsT=VN[kt][:, h * 64:(h + 1) * 64],
                            rhs=ej,
                            start=False, stop=(last and j == 1),
                            skip_group_check=True)
                        nc.tensor.matmul(
                            sums[32 * h:32 * (h + 1), :],
                            lhsT=ones[:],
                            rhs=ej,
                            tile_position=(0, 32 * h),
                            start=False, stop=(last and j == 1),
                            skip_group_check=True)
            rec = recp.tile([128, TCH], f32, tag="rec", name="rec")
            nc.vector.reciprocal(rec[:], sums[:])
            ot_sb = []
            for pr in range(2):
                osb = otsbp.tile([128, TCH], f32r, tag="otsb", name="otsb")
                for hh in range(2):
                    h = pr * 2 + hh
                    for half in range(2):
                        rows = slice(hh * 64 + half * 32,
                                     hh * 64 + half * 32 + 32)
                        nc.vector.tensor_mul(
                            osb[rows, :], otp[pr][rows, :],
                            rec[32 * h:32 * h + 32, :])
                ot_sb.append(osb)
            for ct in range(NCC):
                pp = prjps.tile([128, TCH], f32, tag="prj", name="prj")
                for dc in range(2):
                    nc.tensor.matmul(
                        pp[:],
                        lhsT=r(wot[dc][:, ct * 128:(ct + 1) * 128]),
                        rhs=r(ot_sb[dc][:]),
                        start=(dc == 0), stop=(dc == 1))
                ob = outsbp.tile([128, TCH], f32, tag="ob", name="ob")
                nc.vector.tensor_scalar_add(
                    ob[:], pp[:], bias["bos"][:, ct:ct + 1])
                nc.sync.dma_start(
                    io["out_t"][ct * 128:(ct + 1) * 128, qcols], ob[:])


def build_nc(reps=1):
    from contextlib import ExitStack

    import concourse.tile as tile
    from concourse import bacc, mybir

    f32 = mybir.dt.float32
    nc = bacc.Bacc("TRN2", target_bir_lowering=False, debug=False,
                   num_devices=NCORES)
    io = {}
    for name in ("xq", "xk", "xv"):
        io[name] = nc.dram_tensor(name, [T, C], f32, kind="ExternalInput").ap()
    f32r = mybir.dt.float32r
    for name in ("wqt", "wkt", "wvt"):
        io[name] = nc.dram_tensor(name, [C, DS], f32r,
                                  kind="ExternalInput").ap()
    io["wot"] = nc.dram_tensor("wot", [DS, C], f32r, kind="ExternalInput").ap()
    for name in ("bqs", "bks", "bvs"):
        io[name] = nc.dram_tensor(name, [DS, 1], f32, kind="ExternalInput").ap()
    io["bos"] = nc.dram_tensor("bos", [C, 1], f32, kind="ExternalInput").ap()
    io["ident"] = nc.dram_tensor("ident", [128, 128], f32,
                                 kind="ExternalInput").ap()
    io["out_t"] = nc.dram_tensor("out_t", [C, T], f32,
                                 kind="ExternalOutput").ap()

    with tile.TileContext(nc) as tc:
        if reps == 1:
            with ExitStack() as ctx:
                _emit(ctx, tc, io)
        else:
            with tc.For_i(0, reps, 1):
                with ExitStack() as ctx:
                    _emit(ctx, tc, io)
    nc.compile()
    return nc


def get_nc():
    global _NC_CACHE
    if _NC_CACHE is None:
        _NC_CACHE = build_nc()
    return _NC_CACHE


def tf32_round(x):
    """Round fp32 to tf32 (10-bit mantissa, round-to-nearest-even)."""
    u = np.ascontiguousarray(x, np.float32).view(np.uint32)
    u = (u + 0xFFF + ((u >> 13) & 1)) & np.uint32(0xFFFFE000)
    return u.view(np.float32)


def make_in_maps(q, k, v, Wq, bq, Wk, bk, Wv, bv, Wo, bo):
    q, k, v = (np.asarray(x, np.float32) for x in (q, k, v))
    Wq, Wk, Wv, Wo = (np.asarray(x, np.float32) for x in (Wq, Wk, Wv, Wo))
    bq, bk, bv, bo = (np.asarray(x, np.float32) for x in (bq, bk, bv, bo))
    ident = np.eye(128, dtype=np.float32)
    zeros_c = np.zeros((C, 1), np.float32)
    in_maps = []
    for core in range(NCORES):
        b, g = divmod(core, GROUPS)
        sl = slice(g * DS, (g + 1) * DS)
        in_maps.append({
            "xq": np.ascontiguousarray(q[b]),
            "xk": np.ascontiguousarray(k[b]),
            "xv": np.ascontiguousarray(v[b]),
            "wqt": tf32_round(np.ascontiguousarray(Wq[sl, :].T)),
            "wkt": tf32_round(np.ascontiguousarray(Wk[sl, :].T)),
            "wvt": tf32_round(np.ascontiguousarray(Wv[sl, :].T)),
            "wot": tf32_round(np.ascontiguousarray(Wo[:, sl].T)),
            "bqs": np.ascontiguousarray(bq[sl].reshape(DS, 1)),
            "bks": np.ascontiguousarray(bk[sl].reshape(DS, 1)),
            "bvs": np.ascontiguousarray(bv[sl].reshape(DS, 1)),
            "bos": (np.ascontiguousarray(bo.reshape(C, 1))
                    if g == 0 else zeros_c),
            "ident": ident,
        })
    return in_maps


def combine(results):
    out = np.zeros((B, T, C), np.float32)
    for core in range(NCORES):
        b, _ = divmod(core, GROUPS)
        out[b] += results[core]["out_t"].T
    return out


def kernel(q, k, v, Wq, bq, Wk, bk, Wv, bv, Wo, bo):
    from concourse.bass_utils import run_bass_kernel_spmd

    nc = get_nc()
    in_maps = make_in_maps(q, k, v, Wq, bq, Wk, bk, Wv, bv, Wo, bo)
    res = run_bass_kernel_spmd(nc, in_maps, core_ids=list(range(NCORES)))
    return combine(res.results)


# revision 25
# speedup vs baseline: 1.2156x; 1.2156x over previous
"""MultiHeadAttention Trainium2 kernel (8-core SPMD).

Problem: B=2, T=2048, C=1024, H=16 heads, D=64.
  out = softmax((q Wq^T + bq)(k Wk^T + bk)^T / sqrt(D)) (v Wv^T + bv) Wo^T + bo

Sharding: core c -> (batch b = c // 4, head-group g = c % 4).  Each core
computes 4 heads (a 256-wide slice of the projection space) of one batch
element, including its partial contribution to the row-sharded output
projection.  The host sums the 4 partial outputs per batch and adds bo
(bo itself is folded on-device into the g==0 core's partial).

Per-core dataflow (all matmuls in float32r = tf32, fp32 accumulate):
  - PE-transpose q/k/v tiles to get channel-major activations (contraction
    over C needs C on partitions).
  - QT/KT/VT = W_s @ x^T  ([256, T] channel-major), bias folded in.
  - V_nat[k, d] from VT via PE transposes (needed as AV stationary operand).
  - S^T[k, q] = K_h Q_h^T per head (scores transposed -> no P transpose
    before AV); exp via ScalarE with scale=1/8 folded in; no max
    subtraction (|scores/8| ~ 2 for these inputs, exp is safe in fp32).
  - Row sums via ones-matmul (32 replicated rows per head), softmax
    normalization deferred to the [256, T] attention output.
  - partial^T[co, t] = Wo_s^T.T @ O^T accumulated over the 256 head dims.
"""

import numpy as np

B, T, C, H, D = 2, 2048, 1024, 16, 64
NCORES = 8
GROUPS = 4              # head-groups == cores per batch element
HG = H // GROUPS        # heads per core
DS = HG * D             # per-core projection slice width (256)
TCH = 512               # token chunk (psum bank = 512 fp32)
NTCH = T // TCH         # 4
NCC = C // 128          # 8 contraction chunks
NKT = T // 128          # 16 key tiles
SCALE = float(D) ** -0.5

_NC_CACHE = None

# timing probes: 0=full, 1=stage A only, 2=A+scores+exp, 3=A+B w/o sums,
# 4=full w/o out-proj
PROBE = 0
# dtype of the attention-probability path (es tiles + V-natural weights):
# "f32r" (tf32, best accuracy) or "bf16" (halves PE weight-load time)
AV_DT = "f32r"


def _emit(ctx, tc, io):
    import concourse.bass as bass
    from concourse import mybir

    nc = tc.nc
    f32 = mybir.dt.float32
    f32r = mybir.dt.float32r
    bf16 = mybir.dt.bfloat16
    EXP = mybir.ActivationFunctionType.Exp

    persist = ctx.enter_context(tc.tile_pool(name="persist", bufs=1))

    def ptile(tag, shape, dt=f32):
        return persist.tile(shape, dt, tag=tag, name=tag)

    # --- persistent SBUF tensors ---------------------------------------
    ident = ptile("ident", [128, 128])
    nc.sync.dma_start(ident[:], io["ident"][:, :])
    ones_f = ptile("ones_f", [128, 64])
    nc.vector.memset(ones_f[:], 1.0)

    wsb = {}
    for name in ("wqt", "wkt", "wvt"):
        tiles = []
        for c in range(NCC):
            t_ = ptile(f"{name}{c}", [128, DS], f32r)
            nc.scalar.dma_start(t_[:], io[name][c * 128:(c + 1) * 128, :])
            tiles.append(t_)
        wsb[name] = tiles
    wot = []
    for dc in range(2):
        t_ = ptile(f"wot{dc}", [128, C], f32r)
        nc.scalar.dma_start(t_[:], io["wot"][dc * 128:(dc + 1) * 128, :])
        wot.append(t_)

    bias = {}
    for name, width in (("bqs", 2), ("bks", 2), ("bvs", 2), ("bos", 8)):
        t_ = ptile(name, [128, width])
        nc.scalar.dma_start(
            t_[:], io[name].rearrange("(a p) o -> p (a o)", p=128))
        bias[name] = t_

    QT = [ptile(f"qt{i}", [128, T], f32r) for i in range(2)]
    KT = [ptile(f"kt{i}", [128, T], f32r) for i in range(2)]
    # V natural [k, head*(64 V + 64 ones)]: the ones columns make each
    # head's AV matmul also produce the softmax denominator (replicated
    # across psum rows 64-127)
    av_dt = f32r if AV_DT == "f32r" else bf16
    VN = [ptile(f"vn{i}", [128, 4 * 128], av_dt) for i in range(NKT)]

    probe = PROBE

    # --- stage A: transpose + project q, k, v --------------------------
    with tc.tile_pool(name="vt", bufs=1) as vtp, \
         tc.tile_pool(name="nat", bufs=3) as natp, \
         tc.tile_pool(name="xtsb", bufs=10) as xtsbp, \
         tc.tile_pool(name="xtps", bufs=2, space="PSUM") as xtps, \
         tc.tile_pool(name="projps", bufs=2, space="PSUM") as projps:

        VT = [vtp.tile([128, T], f32r, tag=f"vt{i}", name="vt")
              for i in range(2)]

        def stage_a(xname, wname, bname, XT, tci, ring):
            nat = natp.tile([128, 4 * C], f32, tag="nat", name="nat")
            src_ap = io[xname][tci * TCH:(tci + 1) * TCH, :].rearrange(
                "(j p) c -> p j c", p=128)
            dst_ap = nat[:].rearrange("p (j c) -> p j c", j=4)
            eng = nc.sync if ring % 2 == 0 else nc.scalar
            eng.dma_start(dst_ap, src_ap)
            xts = []
            for c in range(NCC):
                ps = xtps.tile([128, TCH], f32, tag="xt", name="xtps")
                for j in range(4):
                    nc.tensor.matmul(
                        ps[:, j * 128:(j + 1) * 128],
                        lhsT=nat[:, j * C + c * 128:j * C + (c + 1) * 128],
                        rhs=ident[:],
                        is_transpose=True,
                        start=(j == 0), stop=(j == 3))
                xt = xtsbp.tile([128, TCH], f32r, tag="xt", name="xtsb")
                if c % 2 == 0:
                    nc.scalar.copy(xt[:], ps[:])
                else:
                    nc.vector.tensor_copy(xt[:], ps[:])
                xts.append(xt)
            for co in range(2):
                pj = projps.tile([128, TCH], f32, tag="proj", name="proj")
                for c in range(NCC):
                    nc.tensor.matmul(
                        pj[:],
                        lhsT=wsb[wname][c][:, co * 128:(co + 1) * 128],
                        rhs=xts[c][:],
                        start=(c == 0), stop=(c == NCC - 1))
                nc.vector.tensor_scalar_add(
                    XT[co][:, tci * TCH:(tci + 1) * TCH],
                    pj[:], bias[bname][:, co:co + 1])

        # interleave k/v/q chunks for more independent PE work in flight
        for tci in range(NTCH):
            stage_a("xk", "wkt", "bks", KT, tci, 2 * tci)
            stage_a("xv", "wvt", "bvs", VT, tci, 2 * tci + 1)
        # V natural tiles from VT via PE transposes (before q so the q
        # DMAs prefetch under this PE work)
        for tp in range(NKT // 2):
            ps = xtps.tile([128, TCH], f32, tag="xt", name="xtps")
            for u in range(2):
                tt = 2 * tp + u
                for dc in range(2):
                    q_ = 2 * u + dc
                    nc.tensor.matmul(
                        ps[:, q_ * 128:(q_ + 1) * 128],
                        lhsT=VT[dc][:, tt * 128:(tt + 1) * 128].bitcast(f32),
                        rhs=ident[:],
                        is_transpose=True,
                        start=(q_ == 0), stop=(q_ == 3))
            for u in range(2):
                vn = VN[2 * tp + u]
                src3 = ps[:, u * DS:(u + 1) * DS].rearrange(
                    "p (h d) -> p h d", h=4)
                dst3 = vn[:].rearrange("p (h c) -> p h c", h=4)[:, :, 0:64]
                eng_c = nc.scalar.copy if u == 0 else nc.vector.tensor_copy
                eng_c(dst3, src3)
                dst1 = vn[:].rearrange("p (h c) -> p h c", h=4)[:, :, 64:128]
                for h in range(4):
                    nc.vector.tensor_copy(dst1[:, h, :], ones_f[:])

        for tci in range(NTCH):
            stage_a("xq", "wqt", "bqs", QT, tci, tci)

    # --- stage B/C: attention + output projection ----------------------
    # S pool: bufs=3 of [128, 1024] (6 banks) -> the scores->exp->AV chain
    # pipelines 3 deep; per-head accumulators for ONE pair at a time
    # (2 banks).  Head pairs run as two passes per q-chunk.  The
    # out-projection borrows S slots.
    with tc.tile_pool(name="sps", bufs=3, space="PSUM") as sps, \
         tc.tile_pool(name="otps", bufs=2, space="PSUM") as otps, \
         tc.tile_pool(name="expsb", bufs=4) as expsb, \
         tc.tile_pool(name="otsb", bufs=4) as otsbp, \
         tc.tile_pool(name="recsb", bufs=4) as recp, \
         tc.tile_pool(name="outsb", bufs=3) as outsbp:

        for qc in range(NTCH):
            if probe == 1:
                break
            qcols = slice(qc * TCH, (qc + 1) * TCH)
            ot_sb = []
            for pr in range(2):
                otp = [otps.tile([128, TCH], f32, tag="ot", name="ot")
                       for _ in range(2)]
                for g in range(NKT // 2):
                    first = (g == 0)
                    last = (g == NKT // 2 - 1)
                    for hh in range(2):
                        h = pr * 2 + hh
                        rows = slice(hh * 64, (hh + 1) * 64)
                        S = sps.tile([128, 2 * TCH], f32, tag="s", name="s")
                        for j in range(2):
                            kt = 2 * g + j
                            nc.tensor.matmul(
                                S[:, j * TCH:(j + 1) * TCH],
                                lhsT=KT[pr][rows, kt * 128:(kt + 1) * 128],
                                rhs=QT[pr][rows, qcols],
                                start=True, stop=True)
                        es = expsb.tile([128, 2 * TCH], av_dt, tag="es",
                                        name="es")
                        nc.scalar.activation(es[:], S[:], EXP, scale=SCALE)
                        if probe == 2:
                            continue
                        for j in range(2):
                            kt = 2 * g + j
                            nc.tensor.matmul(
                                otp[hh][:, :],
                                lhsT=VN[kt][:, h * 128:(h + 1) * 128],
                                rhs=es[:, j * TCH:(j + 1) * TCH],
                                start=(first and j == 0),
                                stop=(last and j == 1))
                if probe == 2:
                    continue
                # normalize: psum rows 64-127 hold the denominator
                osb = otsbp.tile([128, TCH], f32r, tag="otsb", name="otsb")
                for hh in range(2):
                    rec = recp.tile([64, TCH], f32, tag="rec", name="rec")
                    nc.vector.reciprocal(rec[:], otp[hh][64:128, :])
                    nc.vector.tensor_mul(
                        osb[hh * 64:(hh + 1) * 64, :],
                        otp[hh][0:64, :], rec[:])
                ot_sb.append(osb)
            if probe == 2:
                continue
            if probe == 4:
                continue
            for ct in range(NCC):
                pp = sps.tile([128, TCH], f32, tag="s", name="prj")
                for dc in range(2):
                    nc.tensor.matmul(
                        pp[:],
                        lhsT=wot[dc][:, ct * 128:(ct + 1) * 128],
                        rhs=ot_sb[dc][:],
                        start=(dc == 0), stop=(dc == 1))
                ob = outsbp.tile([128, TCH], f32, tag="ob", name="ob")
                nc.vector.tensor_scalar_add(
                    ob[:], pp[:], bias["bos"][:, ct:ct + 1])
                nc.sync.dma_start(
                    io["out_t"][ct * 128:(ct + 1) * 128, qcols], ob[:])


def build_nc(reps=1):
    from contextlib import ExitStack

    import concourse.tile as tile
    from concourse import bacc, mybir

    f32 = mybir.dt.float32
    nc = bacc.Bacc("TRN2", target_bir_lowering=False, debug=False,
                   num_devices=NCORES)
    io = {}
    for name in ("xq", "xk", "xv"):
        io[name] = nc.dram_tensor(name, [T, C], f32, kind="ExternalInput").ap()
    f32r = mybir.dt.float32r
    for name in ("wqt", "wkt", "wvt"):
        io[name] = nc.dram_tensor(name, [C, DS], f32r,
                                  kind="ExternalInput").ap()
    io["wot"] = nc.dram_tensor("wot", [DS, C], f32r, kind="ExternalInput").ap()
    for name in ("bqs", "bks", "bvs"):
        io[name] = nc.dram_tensor(name, [DS, 1], f32, kind="ExternalInput").ap()
    io["bos"] = nc.dram_tensor("bos", [C, 1], f32, kind="ExternalInput").ap()
    io["ident"] = nc.dram_tensor("ident", [128, 128], f32,
                                 kind="ExternalInput").ap()
    io["out_t"] = nc.dram_tensor("out_t", [C, T], f32,
                                 kind="ExternalOutput").ap()

    with tile.TileContext(nc) as tc:
        if reps == 1:
            with ExitStack() as ctx:
                _emit(ctx, tc, io)
        else:
            with tc.For_i(0, reps, 1):
                with ExitStack() as ctx:
                    _emit(ctx, tc, io)
    nc.compile()
    return nc


def get_nc():
    global _NC_CACHE
    if _NC_CACHE is None:
        _NC_CACHE = build_nc()
    return _NC_CACHE


def tf32_round(x):
    """Round fp32 to tf32 (10-bit mantissa, round-to-nearest-even)."""
    u = np.ascontiguousarray(x, np.float32).view(np.uint32)
    u = (u + 0xFFF + ((u >> 13) & 1)) & np.uint32(0xFFFFE000)
    return u.view(np.float32)


def make_in_maps(q, k, v, Wq, bq, Wk, bk, Wv, bv, Wo, bo):
    q, k, v = (np.asarray(x, np.float32) for x in (q, k, v))
    Wq, Wk, Wv, Wo = (np.asarray(x, np.float32) for x in (Wq, Wk, Wv, Wo))
    bq, bk, bv, bo = (np.asarray(x, np.float32) for x in (bq, bk, bv, bo))
    ident = np.eye(128, dtype=np.float32)
    zeros_c = np.zeros((C, 1), np.float32)
    in_maps = []
    for core in range(NCORES):
        b, g = divmod(core, GROUPS)
        sl = slice(g * DS, (g + 1) * DS)
        in_maps.append({
            "xq": np.ascontiguousarray(q[b]),
            "xk": np.ascontiguousarray(k[b]),
            "xv": np.ascontiguousarray(v[b]),
            "wqt": tf32_round(np.ascontiguousarray(Wq[sl, :].T)),
            "wkt": tf32_round(np.ascontiguousarray(Wk[sl, :].T)),
            "wvt": tf32_round(np.ascontiguousarray(Wv[sl, :].T)),
            "wot": tf32_round(np.ascontiguousarray(Wo[:, sl].T)),
            "bqs": np.ascontiguousarray(bq[sl].reshape(DS, 1)),
            "bks": np.ascontiguousarray(bk[sl].reshape(DS, 1)),
            "bvs": np.ascontiguousarray(bv[sl].reshape(DS, 1)),
            "bos": (np.ascontiguousarray(bo.reshape(C, 1))
                    if g == 0 else zeros_c),
            "ident": ident,
        })
    return in_maps


def combine(results):
    out = np.zeros((B, T, C), np.float32)
    for core in range(NCORES):
        b, _ = divmod(core, GROUPS)
        out[b] += results[core]["out_t"].T
    return out


def kernel(q, k, v, Wq, bq, Wk, bk, Wv, bv, Wo, bo):
    from concourse.bass_utils import run_bass_kernel_spmd

    nc = get_nc()
    in_maps = make_in_maps(q, k, v, Wq, bq, Wk, bk, Wv, bv, Wo, bo)
    res = run_bass_kernel_spmd(nc, in_maps, core_ids=list(range(NCORES)))
    return combine(res.results)


# revision 26
# speedup vs baseline: 1.2344x; 1.0155x over previous
"""MultiHeadAttention Trainium2 kernel (8-core SPMD).

Problem: B=2, T=2048, C=1024, H=16 heads, D=64.
  out = softmax((q Wq^T + bq)(k Wk^T + bk)^T / sqrt(D)) (v Wv^T + bv) Wo^T + bo

Sharding: core c -> (batch b = c // 4, head-group g = c % 4).  Each core
computes 4 heads (a 256-wide slice of the projection space) of one batch
element, including its partial contribution to the row-sharded output
projection.  The host sums the 4 partial outputs per batch and adds bo
(bo itself is folded on-device into the g==0 core's partial).

Per-core dataflow (all matmuls in float32r = tf32, fp32 accumulate):
  - PE-transpose q/k/v tiles to get channel-major activations (contraction
    over C needs C on partitions).
  - QT/KT/VT = W_s @ x^T  ([256, T] channel-major), bias folded in.
  - V_nat[k, d] from VT via PE transposes (needed as AV stationary operand).
  - S^T[k, q] = K_h Q_h^T per head (scores transposed -> no P transpose
    before AV); exp via ScalarE with scale=1/8 folded in; no max
    subtraction (|scores/8| ~ 2 for these inputs, exp is safe in fp32).
  - Row sums via ones-matmul (32 replicated rows per head), softmax
    normalization deferred to the [256, T] attention output.
  - partial^T[co, t] = Wo_s^T.T @ O^T accumulated over the 256 head dims.
"""

import numpy as np

B, T, C, H, D = 2, 2048, 1024, 16, 64
NCORES = 8
GROUPS = 4              # head-groups == cores per batch element
HG = H // GROUPS        # heads per core
DS = HG * D             # per-core projection slice width (256)
TCH = 512               # token chunk (psum bank = 512 fp32)
NTCH = T // TCH         # 4
NCC = C // 128          # 8 contraction chunks
NKT = T // 128          # 16 key tiles
SCALE = float(D) ** -0.5

_NC_CACHE = None

# timing probes: 0=full, 1=stage A only, 2=A+scores+exp, 3=A+B w/o sums,
# 4=full w/o out-proj
PROBE = 0
# dtype of the attention-probability path (es tiles + V-natural weights):
# "f32r" (tf32, best accuracy) or "bf16" (halves PE weight-load time)
AV_DT = "f32r"


def _emit(ctx, tc, io):
    import concourse.bass as bass
    from concourse import mybir

    nc = tc.nc
    f32 = mybir.dt.float32
    f32r = mybir.dt.float32r
    bf16 = mybir.dt.bfloat16
    EXP = mybir.ActivationFunctionType.Exp

    persist = ctx.enter_context(tc.tile_pool(name="persist", bufs=1))

    def ptile(tag, shape, dt=f32):
        return persist.tile(shape, dt, tag=tag, name=tag)

    # --- persistent SBUF tensors ---------------------------------------
    ident = ptile("ident", [128, 128])
    nc.sync.dma_start(ident[:], io["ident"][:, :])
    ones_f = ptile("ones_f", [128, 64])
    nc.vector.memset(ones_f[:], 1.0)

    wsb = {}
    for name in ("wqt", "wkt", "wvt"):
        tiles = []
        for c in range(NCC):
            t_ = ptile(f"{name}{c}", [128, DS], f32r)
            nc.scalar.dma_start(t_[:], io[name][c * 128:(c + 1) * 128, :])
            tiles.append(t_)
        wsb[name] = tiles
    wot = []
    for dc in range(2):
        t_ = ptile(f"wot{dc}", [128, C], f32r)
        nc.scalar.dma_start(t_[:], io["wot"][dc * 128:(dc + 1) * 128, :])
        wot.append(t_)

    bias = {}
    for name, width in (("bqs", 2), ("bks", 2), ("bvs", 2), ("bos", 8)):
        t_ = ptile(name, [128, width])
        nc.scalar.dma_start(
            t_[:], io[name].rearrange("(a p) o -> p (a o)", p=128))
        bias[name] = t_

    QT = [ptile(f"qt{i}", [128, T], f32r) for i in range(2)]
    KT = [ptile(f"kt{i}", [128, T], f32r) for i in range(2)]
    # V natural [k, head*(64 V + 64 ones)]: the ones columns make each
    # head's AV matmul also produce the softmax denominator (replicated
    # across psum rows 64-127)
    av_dt = f32r if AV_DT == "f32r" else bf16
    VN = [ptile(f"vn{i}", [128, 4 * 128], av_dt) for i in range(NKT)]

    probe = PROBE

    # --- stage A: transpose + project q, k, v --------------------------
    with tc.tile_pool(name="vt", bufs=1) as vtp, \
         tc.tile_pool(name="nat", bufs=3) as natp, \
         tc.tile_pool(name="xtsb", bufs=10) as xtsbp, \
         tc.tile_pool(name="xtps", bufs=3, space="PSUM") as xtps, \
         tc.tile_pool(name="projps", bufs=3, space="PSUM") as projps:

        VT = [vtp.tile([128, T], f32r, tag=f"vt{i}", name="vt")
              for i in range(2)]

        def stage_a(xname, wname, bname, XT, tci, ring):
            nat = natp.tile([128, 4 * C], f32, tag="nat", name="nat")
            src_ap = io[xname][tci * TCH:(tci + 1) * TCH, :].rearrange(
                "(j p) c -> p j c", p=128)
            dst_ap = nat[:].rearrange("p (j c) -> p j c", j=4)
            eng = nc.sync if ring % 2 == 0 else nc.scalar
            eng.dma_start(dst_ap, src_ap)
            xts = []
            for c in range(NCC):
                ps = xtps.tile([128, TCH], f32, tag="xt", name="xtps")
                for j in range(4):
                    nc.tensor.matmul(
                        ps[:, j * 128:(j + 1) * 128],
                        lhsT=nat[:, j * C + c * 128:j * C + (c + 1) * 128],
                        rhs=ident[:],
                        is_transpose=True,
                        start=(j == 0), stop=(j == 3))
                xt = xtsbp.tile([128, TCH], f32r, tag="xt", name="xtsb")
                if c % 2 == 0:
                    nc.scalar.copy(xt[:], ps[:])
                else:
                    nc.vector.tensor_copy(xt[:], ps[:])
                xts.append(xt)
            for co in range(2):
                pj = projps.tile([128, TCH], f32, tag="proj", name="proj")
                for c in range(NCC):
                    nc.tensor.matmul(
                        pj[:],
                        lhsT=wsb[wname][c][:, co * 128:(co + 1) * 128],
                        rhs=xts[c][:],
                        start=(c == 0), stop=(c == NCC - 1))
                nc.vector.tensor_scalar_add(
                    XT[co][:, tci * TCH:(tci + 1) * TCH],
                    pj[:], bias[bname][:, co:co + 1])

        # interleave k/v/q chunks for more independent PE work in flight
        for tci in range(NTCH):
            stage_a("xk", "wkt", "bks", KT, tci, 2 * tci)
            stage_a("xv", "wvt", "bvs", VT, tci, 2 * tci + 1)
        for tci in range(NTCH):
            stage_a("xq", "wqt", "bqs", QT, tci, tci)

        # V natural tiles from VT via PE transposes
        for tp in range(NKT // 2):
            ps = xtps.tile([128, TCH], f32, tag="xt", name="xtps")
            for u in range(2):
                tt = 2 * tp + u
                for dc in range(2):
                    q_ = 2 * u + dc
                    nc.tensor.matmul(
                        ps[:, q_ * 128:(q_ + 1) * 128],
                        lhsT=VT[dc][:, tt * 128:(tt + 1) * 128].bitcast(f32),
                        rhs=ident[:],
                        is_transpose=True,
                        start=(q_ == 0), stop=(q_ == 3))
            for u in range(2):
                vn = VN[2 * tp + u]
                src3 = ps[:, u * DS:(u + 1) * DS].rearrange(
                    "p (h d) -> p h d", h=4)
                dst3 = vn[:].rearrange("p (h c) -> p h c", h=4)[:, :, 0:64]
                eng_c = nc.scalar.copy if u == 0 else nc.vector.tensor_copy
                eng_c(dst3, src3)
                dst1 = vn[:].rearrange("p (h c) -> p h c", h=4)[:, :, 64:128]
                for h in range(4):
                    nc.vector.tensor_copy(dst1[:, h, :], ones_f[:])

    # --- stage B/C: attention + output projection ----------------------
    # S pool: bufs=3 of [128, 1024] (6 banks) -> the scores->exp->AV chain
    # pipelines 3 deep; per-head accumulators for ONE pair at a time
    # (2 banks).  Head pairs run as two passes per q-chunk.  The
    # out-projection borrows S slots.
    with tc.tile_pool(name="sps", bufs=3, space="PSUM") as sps, \
         tc.tile_pool(name="otps", bufs=2, space="PSUM") as otps, \
         tc.tile_pool(name="expsb", bufs=4) as expsb, \
         tc.tile_pool(name="otsb", bufs=4) as otsbp, \
         tc.tile_pool(name="recsb", bufs=4) as recp, \
         tc.tile_pool(name="outsb", bufs=3) as outsbp:

        for qc in range(NTCH):
            if probe == 1:
                break
            qcols = slice(qc * TCH, (qc + 1) * TCH)
            ot_sb = []
            for pr in range(2):
                otp = [otps.tile([128, TCH], f32, tag="ot", name="ot")
                       for _ in range(2)]
                for g in range(NKT // 2):
                    first = (g == 0)
                    last = (g == NKT // 2 - 1)
                    for hh in range(2):
                        h = pr * 2 + hh
                        rows = slice(hh * 64, (hh + 1) * 64)
                        S = sps.tile([128, 2 * TCH], f32, tag="s", name="s")
                        for j in range(2):
                            kt = 2 * g + j
                            nc.tensor.matmul(
                                S[:, j * TCH:(j + 1) * TCH],
                                lhsT=KT[pr][rows, kt * 128:(kt + 1) * 128],
                                rhs=QT[pr][rows, qcols],
                                start=True, stop=True)
                        es = expsb.tile([128, 2 * TCH], av_dt, tag="es",
                                        name="es")
                        nc.scalar.activation(es[:], S[:], EXP, scale=SCALE)
                        if probe == 2:
                            continue
                        for j in range(2):
                            kt = 2 * g + j
                            nc.tensor.matmul(
                                otp[hh][:, :],
                                lhsT=VN[kt][:, h * 128:(h + 1) * 128],
                                rhs=es[:, j * TCH:(j + 1) * TCH],
                                start=(first and j == 0),
                                stop=(last and j == 1))
                if probe == 2:
                    continue
                # normalize: psum rows 64-127 hold the denominator
                osb = otsbp.tile([128, TCH], f32r, tag="otsb", name="otsb")
                for hh in range(2):
                    rec = recp.tile([64, TCH], f32, tag="rec", name="rec")
                    nc.vector.reciprocal(rec[:], otp[hh][64:128, :])
                    nc.vector.tensor_mul(
                        osb[hh * 64:(hh + 1) * 64, :],
                        otp[hh][0:64, :], rec[:])
                ot_sb.append(osb)
            if probe == 2:
                continue
            if probe == 4:
                continue
            for ct in range(NCC):
                pp = sps.tile([128, TCH], f32, tag="s", name="prj")
                for dc in range(2):
                    nc.tensor.matmul(
                        pp[:],
                        lhsT=wot[dc][:, ct * 128:(ct + 1) * 128],
                        rhs=ot_sb[dc][:],
                        start=(dc == 0), stop=(dc == 1))
                ob = outsbp.tile([128, TCH], f32, tag="ob", name="ob")
                nc.vector.tensor_scalar_add(
                    ob[:], pp[:], bias["bos"][:, ct:ct + 1])
                nc.sync.dma_start(
                    io["out_t"][ct * 128:(ct + 1) * 128, qcols], ob[:])


def build_nc(reps=1):
    from contextlib import ExitStack

    import concourse.tile as tile
    from concourse import bacc, mybir

    f32 = mybir.dt.float32
    nc = bacc.Bacc("TRN2", target_bir_lowering=False, debug=False,
                   num_devices=NCORES)
    io = {}
    for name in ("xq", "xk", "xv"):
        io[name] = nc.dram_tensor(name, [T, C], f32, kind="ExternalInput").ap()
    f32r = mybir.dt.float32r
    for name in ("wqt", "wkt", "wvt"):
        io[name] = nc.dram_tensor(name, [C, DS], f32r,
                                  kind="ExternalInput").ap()
    io["wot"] = nc.dram_tensor("wot", [DS, C], f32r, kind="ExternalInput").ap()
    for name in ("bqs", "bks", "bvs"):
        io[name] = nc.dram_tensor(name, [DS, 1], f32, kind="ExternalInput").ap()
    io["bos"] = nc.dram_tensor("bos", [C, 1], f32, kind="ExternalInput").ap()
    io["ident"] = nc.dram_tensor("ident", [128, 128], f32,
                                 kind="ExternalInput").ap()
    io["out_t"] = nc.dram_tensor("out_t", [C, T], f32,
                                 kind="ExternalOutput").ap()

    with tile.TileContext(nc) as tc:
        if reps == 1:
            with ExitStack() as ctx:
                _emit(ctx, tc, io)
        else:
            with tc.For_i(0, reps, 1):
                with ExitStack() as ctx:
                    _emit(ctx, tc, io)
    nc.compile()
    return nc


def get_nc():
    global _NC_CACHE
    if _NC_CACHE is None:
        _NC_CACHE = build_nc()
    return _NC_CACHE


def tf32_round(x):
    """Round fp32 to tf32 (10-bit mantissa, round-to-nearest-even)."""
    u = np.ascontiguousarray(x, np.float32).view(np.uint32)
    u = (u + 0xFFF + ((u >> 13) & 1)) & np.uint32(0xFFFFE000)
    return u.view(np.float32)


def make_in_maps(q, k, v, Wq, bq, Wk, bk, Wv, bv, Wo, bo):
    q, k, v = (np.asarray(x, np.float32) for x in (q, k, v))
    Wq, Wk, Wv, Wo = (np.asarray(x, np.float32) for x in (Wq, Wk, Wv, Wo))
    bq, bk, bv, bo = (np.asarray(x, np.float32) for x in (bq, bk, bv, bo))
    ident = np.eye(128, dtype=np.float32)
    zeros_c = np.zeros((C, 1), np.float32)
    in_maps = []
    for core in range(NCORES):
        b, g = divmod(core, GROUPS)
        sl = slice(g * DS, (g + 1) * DS)
        in_maps.append({
            "xq": np.ascontiguousarray(q[b]),
            "xk": np.ascontiguousarray(k[b]),
            "xv": np.ascontiguousarray(v[b]),
            "wqt": tf32_round(np.ascontiguousarray(Wq[sl, :].T)),
            "wkt": tf32_round(np.ascontiguousarray(Wk[sl, :].T)),
            "wvt": tf32_round(np.ascontiguousarray(Wv[sl, :].T)),
            "wot": tf32_round(np.ascontiguousarray(Wo[:, sl].T)),
            "bqs": np.ascontiguousarray(bq[sl].reshape(DS, 1)),
            "bks": np.ascontiguousarray(bk[sl].reshape(DS, 1)),
            "bvs": np.ascontiguousarray(bv[sl].reshape(DS, 1)),
            "bos": (np.ascontiguousarray(bo.reshape(C, 1))
                    if g == 0 else zeros_c),
            "ident": ident,
        })
    return in_maps


def combine(results):
    out = np.zeros((B, T, C), np.float32)
    for core in range(NCORES):
        b, _ = divmod(core, GROUPS)
        out[b] += results[core]["out_t"].T
    return out


def kernel(q, k, v, Wq, bq, Wk, bk, Wv, bv, Wo, bo):
    from concourse.bass_utils import run_bass_kernel_spmd

    nc = get_nc()
    in_maps = make_in_maps(q, k, v, Wq, bq, Wk, bk, Wv, bv, Wo, bo)
    res = run_bass_kernel_spmd(nc, in_maps, core_ids=list(range(NCORES)))
    return combine(res.results)
